# revision 43
# baseline (speedup 1.0000x reference)
"""Trainium2 Bass kernel for nn_LowPassFilter (StyleGAN2-style upfirdn2d).

Semantics (matches reference):
  out = upfirdn2d(x, kernel, up=2, down=1, pad=5)
  x: [8, 64, 256, 256] f32, kernel: [12, 12] f32 -> out: [8, 64, 511, 511] f32

  out[n,c,i,j] = sum_{ky,kx} w[ky,kx] * xup[i+ky-5, j+kx-5]
  with w = flip(kernel), xup[2m] = x[m], xup[odd] = 0.
  Equivalently out[i,j] = sum_{a,b} x[a,b] * B[a,i] * B'[b,j] with banded
  matrices B[a,i] = h[2a+5-i] (0 <= 2a+5-i < 12) for separable kernels
  (h x h'); general kernels are handled via SVD rank decomposition.

Implementation: pure data parallel over batch (8 cores). Per core, per
channel, two TensorEngine passes with the banded matrix as the *moving*
operand (band-limited N ranges), so no transposes are needed:
  pass1: z1[wq,i] = sum_h x[h,wq] * Bc[h,i]     (z1: [W=256, Hout=511])
  pass2: out[i,j] = sum_w z1[w,i] * Br[w,j]     (out: [Hout=511, Wout=511])

End-to-end wall time is dominated by the axon host<->device tunnel
(~75 MB/s up, ~30 MB/s down, partially full-duplex), so the transfer
plan matters more than the on-device schedule:
  * All tunnel I/O is fp16 (quantization l2 ~2e-4 on this problem, two
    orders under the 2e-2 gate): x up 67 MB instead of 134, out down
    267 MB instead of 534.
  * The NEFF is dispatched via _bass_exec_p directly instead of
    run_bass_kernel_spmd's bass2jax redirect. That redirect uploads a
    host-built zero buffer per output (donated so the custom-call
    results are pre-zeroed, for kernels that don't write every output
    element). This kernel writes every element of out, so the results
    can stay uninitialized and the zero upload (another 267-534 MB up)
    is dropped entirely.
  * Channels are processed in CHUNK-sized slices, each its own NEFF
    call on the same compiled executable: the upload of chunk k+1 and
    the fp16->f32 host convert of chunk k-1 overlap the download of
    chunk k on the full-duplex tunnel.
  * LPF_PACK12=1 switches to a 12-bit rowwise fixed-point encoding
    (hi byte + packed nibbles + per-row scale, 1.5B/elem, ~201MB down,
    l2 ~1e-3).  Measured a wash vs fp16: the tunnel saving (~0.5s) is
    offset by the host-side nibble decode, so fp16 stays the default.
  * The jitted sharded callable is cached at module scope, so repeat
    kernel() calls skip retracing.
"""

import os
import time

import numpy as np

_DEBUG = bool(int(os.environ.get("LPF_DEBUG", "0")))

N_CORES = 8
C = 64       # channels per core
CS = 16      # default channels per device dispatch (C % CS == 0)
H = 256
HO = 511
KS = 12
UP = 2
PAD = 5

# Column ranges of the banded matrix reachable from input-row chunk 0
# ([0,128)) vs chunk 1 ([128,256)).  Column i of B draws on rows
# a in [ceil((i-5)/2), floor((i+6)/2)]:
#   chunk0-only: floor((i+6)/2) <= 127  <=> i <= 249
#   chunk1-only: ceil((i-5)/2) >= 128   <=> i >= 260
R0_END = 250     # [0, 250)   chunk0 only
R1_END = 260     # [250, 260) both chunks
# [260, 511) chunk1 only

_CACHE = {}
# Device-resident replicated band matrices, keyed by the raw kernel bytes:
# the 12x12 kernel is the module's weight (nn.Parameter), so keeping its
# transform on device across calls is ordinary weight residency.  The
# activation `x` is uploaded fresh every call.
_BCBR_CACHE = {}

# Kept for test.py compat; the direct-dispatch path has no NTFF profiling,
# so test.py falls back to its steady-state wall-clock bound.
LAST_RESULTS = None

# Reused across calls: a fresh 534MB allocation pays ~1s of first-touch page
# faults inside the convert loop; pages stay warm on repeat calls.
_OUT = None


def _out_buffer() -> np.ndarray:
    global _OUT
    if _OUT is None:
        _OUT = np.empty((N_CORES, C, HO, HO), dtype=np.float32)
    return _OUT


def _band_matrix(h12: np.ndarray) -> np.ndarray:
    """[256, 511] banded matrix B[a, i] = h12[2a + 5 - i] (true-conv taps)."""
    B = np.zeros((H, HO), dtype=np.float64)
    a = np.arange(H)[:, None]
    i = np.arange(HO)[None, :]
    k = 2 * a + PAD - i
    mask = (k >= 0) & (k < KS)
    B[mask] = h12[np.clip(k, 0, KS - 1)][mask]
    return B


def _decompose(kernel: np.ndarray):
    """SVD of the flipped kernel -> list of (hc, hr) rank-1 factor pairs."""
    w = np.flip(kernel.astype(np.float64), (0, 1))
    U, S, Vt = np.linalg.svd(w)
    keep = S > S[0] * 1e-7
    ranks = max(1, int(keep.sum()))
    return [(U[:, r] * S[r], Vt[r, :]) for r in range(ranks)]


def _build_nc(rank: int, cs: int, pack12: bool = False):
    import concourse.mybir as mybir
    from concourse import bacc
    from concourse.tile import TileContext

    f32 = mybir.dt.float32
    f16 = mybir.dt.float16
    u8 = mybir.dt.uint8
    u16 = mybir.dt.uint16
    ALU = mybir.AluOpType
    AX_X = mybir.AxisListType.X
    ACT_COPY = mybir.ActivationFunctionType.Copy

    # pack12: quantize each output row to 12-bit fixed point around a
    # per-row absmax scale and ship hi-byte + packed lo-nibbles + scale
    # (1.5B/elem instead of 2B).  The band is padded 511 -> 512 so nibble
    # pairs and the pad column stay fully initialized (col 511 multiplies a
    # zero band column -> exact 0).
    W = 512 if pack12 else HO

    nc = bacc.Bacc("TRN2", target_bir_lowering=False)
    x_d = nc.dram_tensor("x", [cs, H, H], f16, kind="ExternalInput")
    bc_d = nc.dram_tensor("bc", [rank, 2, 128, W], f16, kind="ExternalInput")
    br_d = nc.dram_tensor("br", [rank, 2, 128, W], f16, kind="ExternalInput")
    if pack12:
        hi_d = nc.dram_tensor("hi", [cs, HO, W], u8, kind="ExternalOutput")
        p_d = nc.dram_tensor("p", [cs, HO, W // 2], u8, kind="ExternalOutput")
        sc_d = nc.dram_tensor("sc", [cs, HO, 1], f32, kind="ExternalOutput")
    else:
        out_d = nc.dram_tensor("out", [cs, HO, HO], f16, kind="ExternalOutput")

    # (column-slice, chunk, start, stop) schedule: regions R0/R1/R2 with the
    # 10-column overlap [250, 260) written by chunk0 then accumulated by
    # chunk1 (PSUM has_written drives accumulate-vs-overwrite).
    def band_mms():
        return [
            (slice(0, R0_END), 0, True, True),
            (slice(R0_END, R1_END), 0, True, False),
            (slice(R0_END, R1_END), 1, False, True),
            (slice(R1_END, W), 1, True, True),
        ]

    # Pass-2 schedule as (cols, chunk, r, start, stop): REGION-major with all
    # rank terms inside one open-accumulate-close group per region.  A PSUM
    # accumulation group must close before another group starts in the same
    # zero region — the rank-outer ordering (all regions of r=0, then r=1)
    # leaves the R0 group pending while R01/R2 groups start, which the
    # hardware resolves by re-zeroing (CoreSim: "already a pending group in
    # that zero region").  For rank 1 this reduces to band_mms() exactly.
    def pass2_mms(rank):
        mms = []
        for r in range(rank):
            mms.append((slice(0, R0_END), 0, r, r == 0, r == rank - 1))
        n = 0
        for ch in (0, 1):
            for r in range(rank):
                mms.append(
                    (slice(R0_END, R1_END), ch, r, n == 0, n == 2 * rank - 1)
                )
                n += 1
        for r in range(rank):
            mms.append((slice(R1_END, W), 1, r, r == 0, r == rank - 1))
        return mms

    with TileContext(nc) as tc:
        with (
            tc.tile_pool(name="const", bufs=1) as constp,
            tc.tile_pool(name="xin", bufs=3) as xp,
            # all 2*rank z1 tiles of a channel stay live from pass 1 until
            # pass 2 consumes them; a smaller ring would make slot reuse wait
            # on pass-2 matmuls that sit behind pass-1 work on the in-order
            # PE queue (deadlock for rank > 1)
            tc.tile_pool(name="z1s", bufs=2 * rank + 2) as z1p,
            tc.tile_pool(name="outs", bufs=6) as outp,
            tc.tile_pool(name="z1ps", bufs=4, space="PSUM") as z1pp,
            tc.tile_pool(name="outps", bufs=3, space="PSUM") as outpp,
        ):
            bc_sb = []
            br_sb = []
            for r in range(rank):
                for t in range(2):
                    bct = constp.tile([128, W], f16, tag=f"bc{r}{t}")
                    nc.sync.dma_start(out=bct, in_=bc_d[r, t])
                    brt = constp.tile([128, W], f16, tag=f"br{r}{t}")
                    nc.sync.dma_start(out=brt, in_=br_d[r, t])
                    bc_sb.append(bct)
                    br_sb.append(brt)

            for c in range(cs):
                x_sb = xp.tile([128, 2, H], f16, tag="x")
                nc.sync.dma_start(
                    out=x_sb, in_=x_d[c].rearrange("(t p) w -> p t w", p=128)
                )

                # pass 1: z1[wq, i] = sum_h x[h, wq] * Bc[h, i], per rank term
                z1_sb = []  # [rank][wt]
                for r in range(rank):
                    z1_r = []
                    for wt in range(2):
                        z1_ps = z1pp.tile([128, W], f32, tag="z1ps")
                        for cols, ch, start, stop in band_mms():
                            nc.tensor.matmul(
                                z1_ps[:, cols],
                                x_sb[:, ch, wt * 128 : (wt + 1) * 128],
                                bc_sb[2 * r + ch][:, cols],
                                start=start,
                                stop=stop,
                            )
                        z1t = z1p.tile([128, W], f16, tag="z1sb")
                        nc.vector.tensor_copy(z1t, z1_ps)
                        z1_r.append(z1t)
                    z1_sb.append(z1_r)

                # pass 2: out[i, j] = sum_w z1[w, i] * Br[w, j]
                for mt in range(4):
                    mrows = 128 if mt < 3 else HO - 3 * 128
                    o_ps = outpp.tile([128, W], f32, tag="ops")
                    for cols, ch, r, start, stop in pass2_mms(rank):
                        nc.tensor.matmul(
                            o_ps[:mrows, cols],
                            z1_sb[r][ch][:, mt * 128 : mt * 128 + mrows],
                            br_sb[2 * r + ch][:, cols],
                            start=start,
                            stop=stop,
                        )
                    rows = slice(mt * 128, mt * 128 + mrows)
                    if not pack12:
                        o_sb = outp.tile([128, W], f16, tag="osb")
                        nc.scalar.copy(o_sb[:mrows], o_ps[:mrows])
                        nc.sync.dma_start(
                            out=out_d[c, rows, :], in_=o_sb[:mrows, 0:HO]
                        )
                        continue

                    # 12-bit rowwise fixed point: qu = o*2039/rowabs + 2048.5
                    # converted to u16 (trunc or round both stay in [9,4088]
                    # and cost at most one quantization step); hi = qu >> 4,
                    # lo = qu & 15, nibble pairs packed lo[2j] | lo[2j+1]<<4.
                    rowabs = outp.tile([128, 1], f32, tag="rowabs")
                    nc.vector.tensor_reduce(
                        rowabs[:mrows], o_ps[:mrows], axis=AX_X,
                        op=ALU.max, apply_absolute_value=True,
                    )
                    nc.vector.tensor_scalar_max(
                        rowabs[:mrows], rowabs[:mrows], 1e-30
                    )
                    rinv = outp.tile([128, 1], f32, tag="rinv")
                    nc.vector.reciprocal(rinv[:mrows], rowabs[:mrows])
                    nc.vector.tensor_scalar_mul(
                        rinv[:mrows], rinv[:mrows], 2039.0
                    )
                    qu16 = outp.tile([128, W], u16, tag="qu16")
                    nc.scalar.activation(
                        qu16[:mrows], o_ps[:mrows], ACT_COPY,
                        scale=rinv[:mrows], bias=2048.5,
                    )
                    hi16 = outp.tile([128, W], u16, tag="hi16")
                    nc.vector.tensor_scalar(
                        hi16[:mrows], qu16[:mrows], 4, None,
                        ALU.logical_shift_right,
                    )
                    hi8 = outp.tile([128, W], u8, tag="hi8")
                    nc.scalar.copy(hi8[:mrows], hi16[:mrows])
                    lo16 = outp.tile([128, W], u16, tag="lo16")
                    nc.vector.tensor_scalar(
                        lo16[:mrows], qu16[:mrows], 15, None, ALU.bitwise_and
                    )
                    lo3 = lo16.rearrange("p (a b) -> p a b", b=2)
                    p16 = outp.tile([128, W // 2], u16, tag="p16")
                    nc.vector.tensor_scalar(
                        p16[:mrows], lo3[:mrows, :, 1], 4, None,
                        ALU.logical_shift_left,
                    )
                    nc.vector.tensor_tensor(
                        p16[:mrows], p16[:mrows], lo3[:mrows, :, 0],
                        ALU.bitwise_or,
                    )
                    p8 = outp.tile([128, W // 2], u8, tag="p8")
                    nc.scalar.copy(p8[:mrows], p16[:mrows])
                    nc.sync.dma_start(out=hi_d[c, rows, :], in_=hi8[:mrows])
                    nc.sync.dma_start(out=p_d[c, rows, :], in_=p8[:mrows])
                    nc.sync.dma_start(out=sc_d[c, rows, :], in_=rowabs[:mrows])
    nc.finalize()
    return nc


def _get_exec(rank: int, cs: int, pack12: bool = False):
    """Build (once) the Bass module and its jitted SPMD dispatcher.

    Mirrors concourse.bass2jax.run_bass_via_pjrt but (a) binds no donated
    zero-output operands — this kernel writes every output element, so the
    custom-call results may start uninitialized — and (b) caches the jitted
    callable so repeat calls skip retracing.
    """
    key = (rank, cs, pack12)
    if key in _CACHE:
        return _CACHE[key]

    import jax
    import concourse.mybir as mybir
    from concourse import bass2jax
    from jax.experimental.shard_map import shard_map
    from jax.sharding import Mesh, NamedSharding, PartitionSpec as P

    bass2jax.install_neuronx_cc_hook()
    nc = _build_nc(rank, cs, pack12)

    in_names = []
    out_names = []
    out_avals = []
    partition_name = nc.partition_id_tensor.name if nc.partition_id_tensor else None
    for alloc in nc.m.functions[0].allocations:
        if not isinstance(alloc, mybir.MemoryLocationSet):
            continue
        name = alloc.memorylocations[0].name
        if alloc.kind == "ExternalInput":
            if name != partition_name:
                in_names.append(name)
        elif alloc.kind == "ExternalOutput":
            out_names.append(name)
            out_avals.append(
                jax.core.ShapedArray(
                    tuple(alloc.tensor_shape), mybir.dt.np(alloc.dtype)
                )
            )
    if partition_name is not None:
        in_names.append(partition_name)

    def _body(*args):
        operands = list(args)
        if partition_name is not None:
            operands.append(bass2jax.partition_id_tensor())
        outs = bass2jax._bass_exec_p.bind(
            *operands,
            out_avals=tuple(out_avals),
            in_names=tuple(in_names),
            out_names=tuple(out_names),
            lowering_input_output_aliases=(),
            sim_require_finite=True,
            sim_require_nnan=True,
            nc=nc,
        )
        return tuple(outs)

    devices = jax.devices()[:N_CORES]
    mesh = Mesh(np.asarray(devices), ("core",))
    n_in = len(in_names) - (1 if partition_name is not None else 0)
    sharded = jax.jit(
        shard_map(
            _body,
            mesh=mesh,
            in_specs=(P("core"),) * n_in,
            out_specs=(P("core"),) * len(out_names),
            check_rep=False,
        ),
        keep_unused=True,
    )
    entry = {
        "sharded": sharded,
        "sharding": NamedSharding(mesh, P("core")),
    }
    _CACHE[key] = entry
    return entry


def kernel(input: np.ndarray, kernel: np.ndarray) -> np.ndarray:
    import jax

    t_pre = time.time()
    x = np.asarray(input)
    factors = _decompose(np.asarray(kernel, dtype=np.float32))
    rank = len(factors)
    cs = int(os.environ.get("LPF_CS", str(CS)))
    # Channel-chunk schedule: two small chunks up front fill the transfer
    # pipe sooner (download of chunk 0 starts after only cs/2 channels of
    # upload+compute), the rest run at the steady chunk size.
    sched = []
    pos = 0
    if bool(int(os.environ.get("LPF_RAMP", "0"))) and cs % 2 == 0 and C > 2 * cs:
        sched = [(0, cs // 2), (cs // 2, cs // 2)]
        pos = cs
    while pos < C:
        sched.append((pos, cs))
        pos += cs

    pack12 = bool(int(os.environ.get("LPF_PACK12", "0")))
    W = 512 if pack12 else HO
    exs = {n: _get_exec(rank, n, pack12) for _, n in set(sched)}
    ex = exs[sched[0][1]]

    wkey = (np.asarray(kernel, dtype=np.float32).tobytes(), W)
    if wkey in _BCBR_CACHE:
        bc_g, br_g = _BCBR_CACHE[wkey]
    else:
        bc = np.zeros((rank, 2, 128, W), dtype=np.float16)
        br = np.zeros((rank, 2, 128, W), dtype=np.float16)
        for r, (hc, hr) in enumerate(factors):
            bc[r, :, :, :HO] = _band_matrix(hc).reshape(2, 128, HO)
            br[r, :, :, :HO] = _band_matrix(hr).reshape(2, 128, HO)
        bc_g = jax.device_put(
            np.broadcast_to(bc, (N_CORES,) + bc.shape).reshape(
                N_CORES * rank, 2, 128, W
            ),
            ex["sharding"],
        )
        br_g = jax.device_put(
            np.broadcast_to(br, (N_CORES,) + br.shape).reshape(
                N_CORES * rank, 2, 128, W
            ),
            ex["sharding"],
        )
        _BCBR_CACHE[wkey] = (bc_g, br_g)

    out = _out_buffer()

    # Dispatch every chunk up front (async), then enqueue all d2h transfers
    # with copy_to_host_async so the axon client streams them back to back;
    # concurrent np.asarray calls from threads measurably degrade tunnel
    # throughput, so the fetch loop below stays single-threaded.
    prefetch = bool(int(os.environ.get("LPF_PREFETCH", "1")))
    t0 = time.time()
    if _DEBUG:
        print(f"  pre: {t0 - t_pre:.2f}s")

    def dispatch(k):
        start, n = sched[k]
        sl = slice(start, start + n)
        xk = x[:, sl].astype(np.float16).reshape(N_CORES * n, H, H)
        outs = exs[n]["sharded"](xk, bc_g, br_g)
        if prefetch:
            for ok in outs:
                ok.copy_to_host_async()
        return outs, sl

    # Fetch shard-by-shard, serially, on this thread; core s / channel-slice
    # k lands in the contiguous block out[s, sl] (cores shard axis 0 of the
    # chunk), and the fp16->f32 convert of each shard runs on one worker so
    # it overlaps the next shard's transfer wait.  Chunk k+1 is dispatched
    # (x slice convert + async upload) while chunk k's download streams, so
    # only chunk 0's dispatch sits in front of the pipeline.
    def convert(s, sl, a):
        out[s, sl] = a

    def decode(s, sl, hi_a, p_a, sc_a):
        # inverse of the device's 12-bit pack (see _build_nc)
        lo = np.empty(hi_a.shape, np.uint8)
        lo[..., 0::2] = p_a & 15
        lo[..., 1::2] = p_a >> 4
        q = (hi_a.astype(np.int16) << 4) | lo
        out[s, sl] = (q[..., :HO].astype(np.float32) - 2048.0) * (
            sc_a * (1.0 / 2039.0)
        )

    import concurrent.futures as cf

    n_chunks = len(sched)
    futs = []
    nxt = dispatch(0)
    with cf.ThreadPoolExecutor(2) as pool:
        for k in range(n_chunks):
            outs_t, sl = nxt
            if k + 1 < n_chunks:
                nxt = dispatch(k + 1)
            t1 = time.time()
            n = sl.stop - sl.start
            if len(outs_t) == 1:
                for shard in outs_t[0].addressable_shards:
                    core = shard.index[0].start // n
                    futs.append(
                        pool.submit(convert, core, sl, np.asarray(shard.data))
                    )
            else:
                percore = {}
                for arr_i, arr in enumerate(outs_t):
                    for shard in arr.addressable_shards:
                        core = shard.index[0].start // n
                        percore.setdefault(core, [None] * 3)[arr_i] = np.asarray(
                            shard.data
                        )
                for core, (hi_a, p_a, sc_a) in percore.items():
                    futs.append(pool.submit(decode, core, sl, hi_a, p_a, sc_a))
            if _DEBUG:
                print(f"  fetch[{sl.start}]: {time.time() - t1:.2f}s")
        for f in futs:
            f.result()
    return out
